# revision 2
# baseline (speedup 1.0000x reference)
"""Trainium2 Bass kernel v2 for BetaBernoulliMixture — product-scan formulation.

post[t] = 1/(1 + K*P[t]), K=(1-w)/w;  P = prod_{tau<t} num2/den2 (range ~e^±15)
  den  = obs ? a1 : b1;     num  = den + (obs ? dal : dbe)
  num2 = num*(t+ab1);       den2 = den*(t+ab2)

Engine split per [128,2048] chunk:
  DVE  - 5 single-pass custom uop programs (~1 elem/cycle each):
         SCAN_ADD (a1 cumsum), DEN_SEL_DBE (select + dbe), NUM2, DEN2,
         SCAN_MULT2 (P = multiplicative scan with the r-multiply fused)
  ACT  - rden2 = exp(-ln(den2)); post = exp(-ln(1 + K*P)) -> bf16;
         b1/b2 bf16 evacuation from PSUM (+dbe for b2)
  PE   - b1 = (ab1+t0)⊗ones + ones⊗iota - I*a1 (fp32) -> PSUM -> DMA
  Pool - idle (its ~2us/semaphore Q7 overhead made it net-negative)
"""

import re
import numpy as np
import ml_dtypes

B, T = 4096, 8192
NCORES = 8
RPC = B // NCORES        # 512 rows per core
P = 128
RC_N = RPC // P          # 4 row chunks
F = 2048                 # t-chunk width
TC_N = T // F            # 4 t chunks
NCONST = 4 + 3 * TC_N    # al1, dal, dbe, dd + (ab1t, ab2t, ab1dbet) per tc
NWAFF = RC_N * TC_N * 2  # (ab1t row, ones) per (rc, tc)

_PROGRAM_CACHE = {}


def _patch_act_tables():
    import concourse.bacc as bacc_mod
    import concourse.hw_specs as hw_specs
    if getattr(bacc_mod, "_act_tables_patched", False):
        return
    orig = hw_specs.get_activation_tables

    def filtered(arch):
        full = orig(arch)
        return {
            name: (funcs if name == "natural_log_exp_and_others" else set())
            for name, funcs in full.items()
        }

    bacc_mod.get_activation_tables = filtered
    bacc_mod._act_tables_patched = True


def _reg_op(name, spec):
    import concourse.dve_ops as dve_ops
    for op in dve_ops.OPS:
        if op.name == name:
            return op
    op = dve_ops.DveOp(name, spec, subdim=False, uops_sha={})
    dve_ops.OPS.append(op)
    dve_ops._SUB_OPCODE_FOR_NAME[name] = (
        dve_ops._CUSTOM_DVE_ROW_BASE + len(dve_ops.OPS) - 1
    )
    dve_ops.CUSTOM_DVE_SPECS[name] = spec
    try:
        op.compile("v3")
    except ValueError as e:
        m = re.search(r"v3: (\w+) ", str(e))
        assert m, str(e)
        object.__setattr__(op, "uops_sha", {"v3": m.group(1)})
        op.compile("v3")
    return op


def _custom_ops():
    from concourse.dve_spec import (
        C0, C1, Idx, Src0, Src1, Spec, select, scan, AluOp)

    def no_ref(*a):
        raise NotImplementedError

    from concourse.dve_ops import (
        RECIPROCAL_APPROX_FAST, RECIP_APPROX_FAST_CONSTS)
    return {
        "recip_fast": RECIPROCAL_APPROX_FAST,
        "rf_consts": RECIP_APPROX_FAST_CONSTS,
        "scan_add": _reg_op("SCAN_ADD_ANT", Spec(
            body=scan(AluOp.ADD, Src0, init=C0), reference=no_ref)),
        "den_sel": _reg_op("DEN_SEL_DBE_ANT", Spec(
            body=select(Src1, Src0, (Idx + C0) - Src0) + C1, reference=no_ref)),
        "num2": _reg_op("NUM2_FROMDENB_ANT", Spec(
            body=(Src0 + Src1 * C0) * (Idx + C1), reference=no_ref)),
        "den2": _reg_op("DEN2_FROMDENB_ANT", Spec(
            body=(Src0 - C0) * (Idx + C1), reference=no_ref)),
        "scan_mult": _reg_op("SCAN_MULT2_ANT", Spec(
            body=scan(AluOp.MULTIPLY, Src0 * Src1, init=C0), reference=no_ref)),
    }


def _build_program(kw: float):
    import concourse.bacc as bacc
    import concourse.mybir as mybir
    from concourse.tile import TileContext

    _patch_act_tables()
    cust = _custom_ops()

    f32 = mybir.dt.float32
    bf16 = mybir.dt.bfloat16
    Alu = mybir.AluOpType
    Act = mybir.ActivationFunctionType

    nc = bacc.Bacc()
    obs_d = nc.dram_tensor("obs", [RPC, T], bf16, kind="ExternalInput")
    rcst_d = nc.dram_tensor("rowconst", [RPC, NCONST], f32, kind="ExternalInput")
    waff_d = nc.dram_tensor("waffs", [NWAFF, P], mybir.dt.float32r, kind="ExternalInput")
    st1_d = nc.dram_tensor("st1", [2, F], mybir.dt.float32r, kind="ExternalInput")
    negI_d = nc.dram_tensor("negI", [P, P], f32, kind="ExternalInput")
    a1_o = nc.dram_tensor("a1_out", [RPC, T], f32, kind="ExternalOutput")
    b1_o = nc.dram_tensor("b1_out", [RPC, T], bf16, kind="ExternalOutput")
    a2_o = nc.dram_tensor("a2_out", [RPC, T], bf16, kind="ExternalOutput")
    b2_o = nc.dram_tensor("b2_out", [RPC, T], bf16, kind="ExternalOutput")
    pm_o = nc.dram_tensor("post_out", [RPC, T], bf16, kind="ExternalOutput")

    with TileContext(nc) as tc:
        with (
            tc.tile_pool(name="consts", bufs=1) as cpool,
            tc.tile_pool(name="rows", bufs=2) as rpool,
            tc.tile_pool(name="work", bufs=2) as wpool,
            tc.psum_pool(name="psb1", bufs=2) as pp1,
        ):
            st1_t = cpool.tile([2, F], mybir.dt.float32r, tag="st1")
            nc.sync.dma_start(st1_t[:], st1_d[:, :])
            negI_t = cpool.tile([P, P], f32, tag="negI")
            nc.sync.dma_start(negI_t[:], negI_d[:, :])

            rows_ts = []
            for rc in range(RC_N):
                r0 = rc * P
                rt = rpool.tile([P, NCONST], f32, tag=f"rows{rc}",
                                name=f"rows{rc}")
                nc.sync.dma_start(rt[:], rcst_d[r0:r0 + P, :])
                rows_ts.append(rt)

            for rc in range(RC_N):
                r0 = rc * P
                rows_t = rows_ts[rc]
                al1 = rows_t[:, 0:1]
                dal = rows_t[:, 1:2]
                dbe = rows_t[:, 2:3]
                dd = rows_t[:, 3:4]

                prev_a1 = prev_P = None
                for tci in range(TC_N):
                    c0 = 4 + 3 * tci
                    ab1t = rows_t[:, c0:c0 + 1]
                    ab2t = rows_t[:, c0 + 1:c0 + 2]
                    t0 = tci * F
                    wrow = (rc * TC_N + tci) * 2

                    obs_t = wpool.tile([P, F], bf16, tag="obs", bufs=3)
                    nc.sync.dma_start(obs_t[:], obs_d[r0:r0 + P, t0:t0 + F])
                    waff_t = rpool.tile([2, P], mybir.dt.float32r, tag="waff")
                    nc.sync.dma_start(waff_t[:], waff_d[wrow:wrow + 2, :])

                    # a1 exclusive scan (col0 = init, scan fills cols 1..F)
                    a1_t = wpool.tile([P, F + 1], f32, tag="a1",
                                      padded_shape=[P, F + 16])
                    a1_init = al1 if tci == 0 else prev_a1[:, F:F + 1]
                    nc.gpsimd.tensor_copy(a1_t[:, 0:1], a1_init)
                    nc.vector._custom_dve(
                        cust["scan_add"], out=a1_t[:, 1:F + 1], in0=obs_t[:],
                        s0=a1_init)
                    a1x = a1_t[:, 0:F]

                    denb_t = wpool.tile([P, F], f32, tag="denb")
                    nc.vector._custom_dve(
                        cust["den_sel"], out=denb_t[:], in0=a1x, in1=obs_t[:],
                        s0=ab1t, s1=dbe)

                    num2_t = wpool.tile([P, F], f32, tag="num2")
                    nc.vector._custom_dve(
                        cust["num2"], out=num2_t[:], in0=denb_t[:],
                        in1=obs_t[:], s0=dd, s1=ab1t)
                    den2_t = wpool.tile([P, F], f32, tag="den2")
                    nc.vector._custom_dve(
                        cust["den2"], out=den2_t[:], in0=denb_t[:],
                        s0=dbe, s1=ab2t)

                    # rden2 = debiased recip_approx_fast (one DVE pass)
                    rden2_t = wpool.tile([P, F], f32, tag="rden2")
                    rfc = cust["rf_consts"]
                    nc.vector._custom_dve(
                        cust["recip_fast"], out=rden2_t[:], in0=den2_t[:],
                        s0=rfc["s0"], s1=rfc["s1"], imm2=2.0000015)

                    # P scan (exclusive via shifted write, init 1.0)
                    P_t = wpool.tile([P, F + 1], f32, tag="P",
                                     padded_shape=[P, F + 16])
                    if tci == 0:
                        nc.gpsimd.memset(P_t[:, 0:1], 1.0)
                        p_init = 1.0
                    else:
                        p_init = prev_P[:, F:F + 1]
                        nc.gpsimd.tensor_copy(P_t[:, 0:1], p_init)
                    nc.vector._custom_dve(
                        cust["scan_mult"], out=P_t[:, 1:F + 1], in0=num2_t[:],
                        in1=rden2_t[:], s0=p_init)

                    # post = exp(-ln(1 + K*P)) -> bf16
                    lp_t = wpool.tile([P, F], f32, tag="lp")
                    nc.scalar.activation(lp_t[:], P_t[:, 0:F], Act.Ln,
                                         scale=kw, bias=1.0)
                    post_t = wpool.tile([P, F], bf16, tag="post")
                    nc.scalar.activation(post_t[:], lp_t[:], Act.Exp,
                                         scale=-1.0)
                    nc.sync.dma_start(pm_o[r0:r0 + P, t0:t0 + F], post_t[:])

                    # a1 fp32 straight from the scan tile; a2 on ACT
                    nc.sync.dma_start(a1_o[r0:r0 + P, t0:t0 + F], a1x)
                    a2bf_t = wpool.tile([P, F], bf16, tag="a2bf")
                    nc.scalar.activation(a2bf_t[:], a1x, Act.Identity, bias=dal)
                    nc.sync.dma_start(a2_o[r0:r0 + P, t0:t0 + F], a2bf_t[:])

                    # b1 on PE: fp32r affine (host-rounded) + fp32 identity
                    b1_ps = pp1.tile([P, F], f32, tag="b1ps")
                    for j in range(0, F, 512):
                        nc.tensor.matmul(b1_ps[:, j:j + 512], waff_t[:],
                                         st1_t[:, j:j + 512],
                                         start=True, stop=False)
                        nc.tensor.matmul(b1_ps[:, j:j + 512], negI_t[:],
                                         a1x[:, j:j + 512],
                                         start=False, stop=True)

                    # b1/b2 -> bf16 via ACT reading PSUM
                    b1bf_t = wpool.tile([P, F], bf16, tag="b1bf")
                    nc.scalar.activation(b1bf_t[:], b1_ps[:], Act.Identity)
                    nc.sync.dma_start(b1_o[r0:r0 + P, t0:t0 + F], b1bf_t[:])
                    b2bf_t = wpool.tile([P, F], bf16, tag="b2bf")
                    nc.scalar.activation(b2bf_t[:], b1_ps[:], Act.Identity,
                                         bias=dbe)
                    nc.sync.dma_start(b2_o[r0:r0 + P, t0:t0 + F], b2bf_t[:])

                    prev_a1, prev_P = a1_t, P_t

    nc.finalize()
    return nc


def _pack_rowconst(alpha1, beta1, alpha2, beta2):
    a1 = alpha1.astype(np.float32)
    b1 = beta1.astype(np.float32)
    a2 = alpha2.astype(np.float32)
    b2 = beta2.astype(np.float32)
    dal = a2 - a1
    dbe = b2 - b1
    cols = [a1, dal, dbe, dal - dbe]
    ab1 = a1 + b1
    ab2 = a2 + b2
    for tci in range(TC_N):
        t0 = np.float32(tci * F)
        cols.extend([ab1 + t0, ab2 + t0, ab1 + t0 + dbe])
    return np.ascontiguousarray(np.stack(cols, axis=1), dtype=np.float32)


def _round_tf32(x):
    xi = x.astype(np.float32).view(np.int32)
    return ((xi + 0x1000) & ~0x1FFF).view(np.float32)


def _pack_waffs(alpha1, beta1, core_r0):
    """[NWAFF, P]: per (rc, tci): (ab1+t0) row then ones row, tf32-ground."""
    ab1 = alpha1.astype(np.float32) + beta1.astype(np.float32)
    ones = np.ones(P, np.float32)
    rows = []
    for rc in range(RC_N):
        r0 = core_r0 + rc * P
        for tci in range(TC_N):
            rows.append(_round_tf32(ab1[r0:r0 + P] + np.float32(tci * F)))
            rows.append(ones)
    return np.ascontiguousarray(np.stack(rows), dtype=np.float32)


def build_in_maps(obs_seq, alpha1, beta1, alpha2, beta2):
    obs_bf = np.asarray(obs_seq).astype(ml_dtypes.bfloat16)
    alpha1 = np.asarray(alpha1)
    beta1 = np.asarray(beta1)
    alpha2 = np.asarray(alpha2)
    beta2 = np.asarray(beta2)
    rowconst = _pack_rowconst(alpha1, beta1, alpha2, beta2)
    st1 = np.ascontiguousarray(
        np.stack([np.ones(F, np.float32), np.arange(F, dtype=np.float32)]),
        dtype=np.float32)
    negI = np.ascontiguousarray((-np.eye(P)).astype(np.float32))
    in_maps = []
    for c in range(NCORES):
        r0 = c * RPC
        in_maps.append({
            "obs": np.ascontiguousarray(obs_bf[r0:r0 + RPC]),
            "rowconst": np.ascontiguousarray(rowconst[r0:r0 + RPC]),
            "waffs": _pack_waffs(alpha1, beta1, r0),
            "st1": st1,
            "negI": negI,
        })
    return in_maps


def kernel(obs_seq, alpha1, beta1, alpha2, beta2, mixweight):
    from concourse.bass_utils import run_bass_kernel_spmd

    w = float(np.float32(mixweight))
    kw = float(np.float32((1.0 - w) / w))
    if kw not in _PROGRAM_CACHE:
        _PROGRAM_CACHE[kw] = _build_program(kw)
    nc = _PROGRAM_CACHE[kw]

    in_maps = build_in_maps(obs_seq, alpha1, beta1, alpha2, beta2)
    res = run_bass_kernel_spmd(nc, in_maps, core_ids=list(range(NCORES)))
    out = np.empty((5, B, T), np.float32)
    names = ["a1_out", "b1_out", "a2_out", "b2_out", "post_out"]
    for c in range(NCORES):
        r0 = c * RPC
        for k, name in enumerate(names):
            out[k, r0:r0 + RPC] = np.asarray(
                res.results[c][name], dtype=np.float32)
    return out
